# revision 1
# baseline (speedup 1.0000x reference)
"""Trainium2 Bass kernel for nn_NeuralALU (batched byte-encoded 32-bit add).

The reference network computes, per batch element, a chain of table-lookup
matmuls + sharp softmaxes (scale=100) over exactly-one-hot byte encodings.
Because the inputs are exact one-hots, the float pipeline collapses to a
discrete algorithm (validated to 0 rel-err on all significant entries):

  a_val, b_val  = argmax of the 256-wide one-hots per byte
  xl = (a%16 + b%16), xh = (a>>4 + b>>4)           per byte, in [0,30]
  carry state c in {0, 0.5, 1}, init 0.5, over 8 nibbles (lo0,hi0,...,hi3):
      add = (c == 1); y = x + add; U = y mod 16; P = (c == 0.5)
      c' = clamp(x + c - 15, 0, 1)
  nibble dist = onehot(U)*(1-P/2) + onehot((U+1) mod 16)*(P/2)
  out byte row [256] = outer(h_dist, l_dist) flattened

Sharding: pure data parallel over the batch dim across 8 NeuronCores.
Per-core: 32 row-tiles of 128 in 2 chunks (extraction + carry chain per
chunk), nibble distributions in 4-tile sub-chunks, outer products fused
over tile pairs. Outers run on GPSIMD except the final sub-chunks, which
use the (by then idle) vector engine to shorten the tail.
"""

import numpy as np

import concourse.bass as bass
import concourse.bacc as bacc
import concourse.mybir as mybir
from concourse.tile import TileContext
from concourse.bass_utils import run_bass_kernel_spmd

N_CORES = 8
B_FULL = 32768
ROWS = B_FULL // N_CORES  # 4096 rows per core
F = 1024  # 4 bytes x 256 one-hot
P = 128
TILES_PER_CHUNK = 16
SUB = 4  # tiles per distribution sub-chunk
TAIL_VEC_SUBS = 2  # last-chunk sub-chunks whose outers run on DVE

FP = mybir.dt.float32
I32 = mybir.dt.int32


def _const_tables():
    k = np.arange(256)
    z = ((k % 16) + 32 * (k // 16)).astype(np.float32)
    # two bytes per dot: second byte's code scaled by 2^10 (sums stay exact
    # in f32: max 990*1024+990 < 2^24)
    ztab2 = np.concatenate([z, z * 1024.0])  # [512]
    ztab2 = np.broadcast_to(ztab2, (P, 512)).copy()
    # padded compare table: iota17b[j] = (j-1) mod 16. eq = [U == iota17b]
    # gives [U==k] at cols 1..16 and [U==15] at col 0, so cols 0..15 are
    # exactly [(U+1) mod 16 == k] -- one compare yields both one-hots.
    i17 = ((np.arange(17) + 15) % 16).astype(np.float32)
    iota17 = np.broadcast_to(i17, (P, 17)).copy()
    return ztab2, iota17


def build_nc(rows=ROWS):
    nt = rows // P
    ntc = min(TILES_PER_CHUNK, nt)
    assert nt % ntc == 0 and ntc % SUB == 0
    n_chunks = nt // ntc
    nsub = ntc // SUB

    # Bacc (not raw Bass): its compile pass legalizes multi-wait sync;
    # this walrus build allows only one embedded wait per instruction.
    nc = bacc.Bacc()
    # a and b are concatenated host-side so each tile needs a single DMA.
    ab_d = nc.declare_dram_parameter("ab", [2 * rows, F], FP, isOutput=False)
    ztab_d = nc.declare_dram_parameter("ztab2", [P, 512], FP, isOutput=False)
    iota_d = nc.declare_dram_parameter("iota17", [P, 17], FP, isOutput=False)
    out_d = nc.declare_dram_parameter("out", [rows, F], FP, isOutput=True)

    ab_v = ab_d[:, :].rearrange("(j t p) f -> t p j f", j=2, p=P)
    # paired output view: [pair u] -> [p, t2, f]
    out2_v = out_d[:, :].rearrange("(u t2 p) f -> u p t2 f", t2=2, p=P)

    AL = mybir.AluOpType

    with TileContext(nc) as tc:
        with (
            tc.tile_pool(name="consts", bufs=1) as cpool,
            tc.tile_pool(name="io", bufs=6) as iopool,
            tc.tile_pool(name="s", bufs=4) as spool,
            tc.tile_pool(name="scratch", bufs=4) as scpool,
            tc.tile_pool(name="arrs", bufs=2) as apool,
            tc.tile_pool(name="dist", bufs=3) as dpool,
            tc.tile_pool(name="outp", bufs=4) as opool,
        ):
            ztab_raw = cpool.tile([P, 512], FP, tag="ztab_raw")
            ztab = cpool.tile([P, 512], FP, tag="ztab")
            iota_raw = cpool.tile([P, 17], FP, tag="iota_raw")
            iota17 = cpool.tile([P, 17], FP, tag="iota17")
            nc.sync.dma_start(ztab_raw[:, :], ztab_d[:, :])
            nc.sync.dma_start(iota_raw[:, :], iota_d[:, :])
            # pre-touch consts on DVE so compute ops only wait on DVE state
            nc.vector.tensor_copy(ztab[:, :], ztab_raw[:, :])
            nc.vector.tensor_copy(iota17[:, :], iota_raw[:, :])

            # out-DMAs of chunk k are emitted after chunk k+1's input DMAs so
            # they never head-of-line block the input stream on the SP queue
            pending_outs = []
            for ch in range(n_chunks):
                t0 = ch * ntc
                z2 = apool.tile([P, 2 * ntc], FP, tag="z2")
                z2_i = apool.tile([P, 2 * ntc], I32, tag="z2i")
                zb_i = apool.tile([P, 4 * ntc], I32, tag="zbi")
                xlo_i = apool.tile([P, 4 * ntc], I32, tag="xloi")
                xhi_i = apool.tile([P, 4 * ntc], I32, tag="xhii")
                xnib = apool.tile([P, 8 * ntc], FP, tag="xnib")
                c_hist = apool.tile([P, 9 * ntc], FP, tag="chist")
                ctmp = apool.tile([P, ntc], FP, tag="ctmp")
                add_all = apool.tile([P, 8 * ntc], FP, tag="add")
                p_all = apool.tile([P, 8 * ntc], FP, tag="pall")
                y_all = apool.tile([P, 8 * ntc], FP, tag="yall")
                wrap = apool.tile([P, 8 * ntc], FP, tag="wrap")
                u_all = apool.tile([P, 8 * ntc], FP, tag="uall")
                w0_all = apool.tile([P, 8 * ntc], FP, tag="w0")
                w1_all = apool.tile([P, 8 * ntc], FP, tag="w1")

                # ---- phase 1: load + s=a+b + byte-pair dots -> z2 ----
                for t in range(ntc):
                    ab_t = iopool.tile([P, 2 * F], FP, tag="ab")
                    ab_tv = ab_t[:, :].rearrange("p (j f) -> p j f", j=2)
                    nc.sync.dma_start(ab_tv, ab_v[t0 + t])
                    s_t = spool.tile([P, F], FP, tag="s")
                    # s on DVE: offloading to gpsimd stalls the dependent dot
                    # ops (DVE stream is FIFO; embedded waits block it), which
                    # measured slower every time despite the freed cycles.
                    nc.vector.tensor_add(s_t[:, :], ab_t[:, 0:F], ab_t[:, F : 2 * F])
                    for i2 in range(2):
                        prod = scpool.tile([P, 512], FP, tag="prod")
                        # accum = dot(s bytes [2i2,2i2+1], ztab2)
                        nc.vector.scalar_tensor_tensor(
                            out=prod[:, :],
                            in0=s_t[:, i2 * 512 : (i2 + 1) * 512],
                            scalar=1.0,
                            in1=ztab[:, :],
                            op0=AL.mult,
                            op1=AL.mult,
                            accum_out=z2[:, i2 * ntc + t : i2 * ntc + t + 1],
                        )
                for u_idx, o2p in pending_outs:
                    nc.sync.dma_start(out2_v[u_idx], o2p[:, :])
                pending_outs = []

                # ---- phase 2: split z2 -> per-byte nibble sums xnib ----
                nc.vector.tensor_copy(z2_i[:, :], z2[:, :])  # f32 -> i32 exact
                zb_v = zb_i[:, :].rearrange("p (i2 par t) -> p i2 par t", par=2, t=ntc)
                z2_v = z2_i[:, :].rearrange("p (i2 t) -> p i2 t", t=ntc)
                nc.vector.tensor_scalar(
                    out=zb_v[:, :, 0, :], in0=z2_v, scalar1=1023, scalar2=None,
                    op0=AL.bitwise_and,
                )
                nc.vector.tensor_scalar(
                    out=zb_v[:, :, 1, :], in0=z2_v, scalar1=10, scalar2=None,
                    op0=AL.logical_shift_right,
                )
                nc.vector.tensor_scalar(
                    out=xlo_i[:, :], in0=zb_i[:, :], scalar1=31, scalar2=None,
                    op0=AL.bitwise_and,
                )
                nc.vector.tensor_scalar(
                    out=xhi_i[:, :], in0=zb_i[:, :], scalar1=5, scalar2=None,
                    op0=AL.logical_shift_right,
                )
                xnib_v = xnib[:, :].rearrange("p (i two t) -> p i two t", two=2, t=ntc)
                nc.vector.tensor_copy(
                    xnib_v[:, :, 0, :],
                    xlo_i[:, :].rearrange("p (i t) -> p i t", t=ntc),
                )
                nc.vector.tensor_copy(
                    xnib_v[:, :, 1, :],
                    xhi_i[:, :].rearrange("p (i t) -> p i t", t=ntc),
                )

                # ---- phase 3: sequential carry chain over 8 nibbles ----
                nc.vector.memset(c_hist[:, 0:ntc], 0.5)
                for n in range(8):
                    x_n = xnib[:, n * ntc : (n + 1) * ntc]
                    c_in = c_hist[:, n * ntc : (n + 1) * ntc]
                    c_out = c_hist[:, (n + 1) * ntc : (n + 2) * ntc]
                    nc.vector.scalar_tensor_tensor(
                        out=ctmp[:, :], in0=x_n, scalar=-15.0, in1=c_in,
                        op0=AL.add, op1=AL.add,
                    )
                    nc.vector.tensor_scalar(
                        out=c_out, in0=ctmp[:, :], scalar1=0.0, scalar2=1.0,
                        op0=AL.max, op1=AL.min,
                    )

                # ---- phase 4: vectorized U/P/weights over all nibbles ----
                c_pre = c_hist[:, 0 : 8 * ntc]
                nc.vector.tensor_scalar(
                    out=add_all[:, :], in0=c_pre, scalar1=0.75, scalar2=None,
                    op0=AL.is_ge,
                )
                nc.vector.tensor_scalar(
                    out=p_all[:, :], in0=c_pre, scalar1=0.5, scalar2=None,
                    op0=AL.is_equal,
                )
                nc.vector.tensor_add(y_all[:, :], xnib[:, :], add_all[:, :])
                nc.vector.tensor_scalar(
                    out=wrap[:, :], in0=y_all[:, :], scalar1=15.5, scalar2=None,
                    op0=AL.is_ge,
                )
                nc.vector.scalar_tensor_tensor(
                    out=u_all[:, :], in0=wrap[:, :], scalar=-16.0, in1=y_all[:, :],
                    op0=AL.mult, op1=AL.add,
                )
                nc.vector.tensor_scalar(
                    out=w1_all[:, :], in0=p_all[:, :], scalar1=0.5, scalar2=None,
                    op0=AL.mult,
                )
                nc.vector.tensor_scalar(
                    out=w0_all[:, :], in0=p_all[:, :], scalar1=-0.5, scalar2=1.0,
                    op0=AL.mult, op1=AL.add,
                )

                # ---- phases 5+6 per sub-chunk: dists then paired outers ----
                u_nv = u_all[:, :].rearrange("p (n t) -> p n t", t=ntc)
                w0_nv = w0_all[:, :].rearrange("p (n t) -> p n t", t=ntc)
                w1_nv = w1_all[:, :].rearrange("p (n t) -> p n t", t=ntc)
                for sb in range(nsub):
                    ts0 = sb * SUB
                    shape17 = [P, 8, SUB, 17]
                    shape16 = [P, 8, SUB, 16]
                    iota_b = iota17[:, None, None, :].broadcast_to(shape17)
                    u_b = u_nv[:, :, ts0 : ts0 + SUB][:, :, :, None].broadcast_to(shape17)
                    w0_b = w0_nv[:, :, ts0 : ts0 + SUB][:, :, :, None].broadcast_to(shape16)
                    w1_b = w1_nv[:, :, ts0 : ts0 + SUB][:, :, :, None].broadcast_to(shape16)
                    eqx = dpool.tile([P, 8 * SUB * 17], FP, tag="eqx")
                    dsub = dpool.tile([P, 8 * SUB * 16], FP, tag="dsub")
                    dtmp = dpool.tile([P, 8 * SUB * 16], FP, tag="dtmp")
                    eqx_v = eqx[:, :].rearrange("p (n t k) -> p n t k", t=SUB, k=17)
                    dsub_v = dsub[:, :].rearrange("p (n t k) -> p n t k", t=SUB, k=16)
                    dtmp_v = dtmp[:, :].rearrange("p (n t k) -> p n t k", t=SUB, k=16)
                    # dist build stays fully on DVE: moving the muls to
                    # gpsimd (cross-engine ping-pong) measured slower.
                    # eqx[.., j] = [U == (j-1) mod 16]:
                    #   cols 1..16 = onehot(U), cols 0..15 = onehot((U+1)%16)
                    nc.vector.tensor_tensor(eqx_v, u_b, iota_b, op=AL.is_equal)
                    nc.vector.tensor_mul(dsub_v, eqx_v[:, :, :, 1:17], w0_b)
                    nc.vector.tensor_mul(dtmp_v, eqx_v[:, :, :, 0:16], w1_b)
                    nc.vector.tensor_add(dsub[:, :], dsub[:, :], dtmp[:, :])

                    dv = dsub[:, :].rearrange(
                        "p (i par t k) -> p i par t k", par=2, t=SUB, k=16
                    )
                    last_subs = (ch == n_chunks - 1) and (sb >= nsub - TAIL_VEC_SUBS)
                    eng = nc.vector if last_subs else nc.gpsimd
                    for tp in range(SUB // 2):
                        tl = tp * 2
                        o2 = opool.tile([P, 2 * F], FP, tag="o2")
                        for t2 in range(2):  # TT allows max 3 free dims
                            o_v = o2[:, t2 * F : (t2 + 1) * F].rearrange(
                                "p (i h k) -> p i h k", h=16, k=16
                            )
                            h_b = dv[:, :, 1, tl + t2, :][:, :, :, None].broadcast_to(
                                [P, 4, 16, 16])
                            l_b = dv[:, :, 0, tl + t2, :][:, :, None, :].broadcast_to(
                                [P, 4, 16, 16])
                            eng.tensor_mul(o_v, h_b, l_b)
                        u_idx = (t0 + ts0 + tl) // 2
                        if ch == n_chunks - 1:
                            nc.sync.dma_start(out2_v[u_idx], o2[:, :])
                        else:
                            pending_outs.append((u_idx, o2))

    nc.finalize()
    return nc


_NC_CACHE = {}
LAST_RESULT = None


def kernel(**inputs) -> np.ndarray:
    global LAST_RESULT
    a = np.ascontiguousarray(np.asarray(inputs["a"], dtype=np.float32)).reshape(B_FULL, F)
    b = np.ascontiguousarray(np.asarray(inputs["b"], dtype=np.float32)).reshape(B_FULL, F)
    ztab2, iota17 = _const_tables()

    if ROWS not in _NC_CACHE:
        _NC_CACHE[ROWS] = build_nc(ROWS)
    nc = _NC_CACHE[ROWS]

    in_maps = []
    for c in range(N_CORES):
        ab = np.concatenate(
            [a[c * ROWS : (c + 1) * ROWS], b[c * ROWS : (c + 1) * ROWS]], axis=0
        )
        in_maps.append({
            "ab": np.ascontiguousarray(ab),
            "ztab2": ztab2,
            "iota17": iota17,
        })
    res = run_bass_kernel_spmd(nc, in_maps, core_ids=list(range(N_CORES)))
    LAST_RESULT = res
    out = np.concatenate([r["out"] for r in res.results], axis=0)
    return out.reshape(B_FULL, 4, 256)

